# revision 1
# baseline (speedup 1.0000x reference)
"""Trainium2 Bass kernel for nn_CCM: per-pixel complex 3x3 conv mask.

Math (per batch element b, sharded 1 batch element per NeuronCore):
  y[t,f] = sum_{c=0..26} m[c,t,f] * (w_{k(c)} * X)[t+i(c)-2, f+j(c)-1]
where c = 9*k + 3*i + j, w_k = v[0,k] + 1j*v[1,k] (cube roots of unity),
X = xr + 1j*xi, zero padded (causal in t: 2 top; symmetric in f: 1,1).

Layout: t = 8*p + tau, partitions p in [0,125), (tau, f) in the free dim,
so every tap shift is a free-dim offset read of padded "U" planes
U_k = w_k * X stored as [125, 10 tau-slots, 259 f-cols] (slots tau=-2..7).
"""

import sys
import numpy as np

sys.path.insert(0, "/opt/trn_rl_repo")

B = 8
C = 27
T = 1000
F = 257
TP = 125          # partitions
TAU = 8           # t = 8*p + tau
NS = 10           # tau slots in U planes: tau in [-2, 8)
FP = 259          # padded f width: f in [-1, 258)
SQ3H = float(np.sqrt(3.0) / 2.0)

_CACHE = {}


def _emit(ctx, tc, m_ap, x_ap, id_ap, y_ap):
    import concourse.mybir as mybir

    nc = tc.nc
    f32 = mybir.dt.float32
    FCS = [(0, 128), (128, 128), (256, 1)]  # f chunks for transposes

    const = ctx.enter_context(tc.tile_pool(name="const", bufs=1))
    planes = ctx.enter_context(tc.tile_pool(name="planes", bufs=1))
    mpool = ctx.enter_context(tc.tile_pool(name="mtiles", bufs=3))
    work = ctx.enter_context(tc.tile_pool(name="work", bufs=3))
    psum = ctx.enter_context(tc.tile_pool(name="psum", bufs=3, space="PSUM"))

    ident = const.tile([128, 128], f32, tag="ident")
    nc.sync.dma_start(ident[:], id_ap)

    # ---- load x in natural layout: [f, (tt, comp)] with tt = t + 2 (2 zero rows)
    xns = []
    for (f0, fw) in FCS:
        xn = const.tile([fw, (T + 2) * 2], f32, tag=f"xn{f0}")
        nc.vector.memset(xn[:, 0:4], 0.0)
        nc.sync.dma_start(
            xn[:, 4:], x_ap[f0:f0 + fw].rearrange("f t c -> f (t c)")
        )
        xns.append(xn)

    # ---- transpose x into blocked padded planes xr, xi: [TP, NS, FP]
    xq = []
    for q in range(2):
        p = planes.tile([TP, NS, FP], f32, tag=f"xq{q}")
        nc.vector.memset(p[:], 0.0)
        xq.append(p)
    for q in range(2):
        for ts in range(NS):  # slot ts corresponds to tau = ts - 2; tt = 8p + ts
            for ci, (f0, fw) in enumerate(FCS):
                pt = psum.tile([TP, 128], f32, tag="tp")
                xn3 = xns[ci].rearrange("f (t c) -> f t c", c=2)
                nc.tensor.transpose(
                    pt[0:TP, 0:fw],
                    xn3[0:fw, ts:ts + TAU * (TP - 1) + 1:TAU, q],
                    ident[0:fw, 0:fw],
                )
                nc.scalar.copy(xq[q][:, ts, 1 + f0:1 + f0 + fw], pt[0:TP, 0:fw])

    # ---- U planes: U_k = w_k * (xr + i xi), w_k = exp(+-2pi i/3), w_0 = 1
    mult = mybir.AluOpType.mult
    add = mybir.AluOpType.add
    sub = mybir.AluOpType.subtract
    t1 = planes.tile([TP, NS, FP], f32, tag="t1")
    t2 = planes.tile([TP, NS, FP], f32, tag="t2")
    ur1 = planes.tile([TP, NS, FP], f32, tag="ur1")
    ui1 = planes.tile([TP, NS, FP], f32, tag="ui1")
    ur2 = planes.tile([TP, NS, FP], f32, tag="ur2")
    ui2 = planes.tile([TP, NS, FP], f32, tag="ui2")
    nc.vector.tensor_scalar_mul(t1[:], xq[1][:], SQ3H)  # xi * s
    nc.vector.tensor_scalar_mul(t2[:], xq[0][:], SQ3H)  # xr * s
    nc.vector.scalar_tensor_tensor(ur1[:], xq[0][:], -0.5, t1[:], op0=mult, op1=sub)
    nc.vector.scalar_tensor_tensor(ui1[:], xq[1][:], -0.5, t2[:], op0=mult, op1=add)
    nc.vector.scalar_tensor_tensor(ur2[:], xq[0][:], -0.5, t1[:], op0=mult, op1=add)
    nc.vector.scalar_tensor_tensor(ui2[:], xq[1][:], -0.5, t2[:], op0=mult, op1=sub)
    U = [(xq[0], xq[1]), (ur1, ui1), (ur2, ui2)]

    # ---- tap loop: acc += m_c * U_k[shifted]
    acc_r = planes.tile([TP, TAU, F], f32, tag="accr")
    acc_i = planes.tile([TP, TAU, F], f32, tag="acci")
    for c in range(C):
        kk, n = divmod(c, 9)
        i, j = divmod(n, 3)
        dt, df = i - 2, j - 1
        mt = mpool.tile([TP, TAU * F], f32, tag="mt")
        nc.sync.dma_start(mt[:], m_ap[c].rearrange("(p t) f -> p (t f)", p=TP))
        m3 = mt.rearrange("p (t f) -> p t f", f=F)
        ur, ui = U[kk]
        urs = ur[:, dt + 2:dt + 2 + TAU, df + 1:df + 1 + F]
        uis = ui[:, dt + 2:dt + 2 + TAU, df + 1:df + 1 + F]
        if c == 0:
            nc.vector.tensor_mul(acc_r[:], m3[:], urs)
            nc.vector.tensor_mul(acc_i[:], m3[:], uis)
        else:
            pr = work.tile([TP, TAU, F], f32, tag="prod")
            nc.vector.tensor_mul(pr[:], m3[:], urs)
            nc.vector.tensor_add(acc_r[:], acc_r[:], pr[:])
            pi = work.tile([TP, TAU, F], f32, tag="prod")
            nc.vector.tensor_mul(pi[:], m3[:], uis)
            nc.vector.tensor_add(acc_i[:], acc_i[:], pi[:])

    # ---- transpose back to [f, (t, comp)] and store
    for ci, (f0, fw) in enumerate(FCS):
        yo = const.tile([fw, T * 2], f32, tag=f"yo{f0}")
        yv = yo.rearrange("f (t c) -> f t c", c=2)
        for comp, acc in ((0, acc_r), (1, acc_i)):
            for ts in range(TAU):
                pt = psum.tile([128, TP], f32, tag="tp2")
                nc.tensor.transpose(
                    pt[0:fw, 0:TP], acc[:, ts, f0:f0 + fw], ident[0:TP, 0:TP]
                )
                nc.scalar.copy(
                    yv[0:fw, ts:ts + TAU * (TP - 1) + 1:TAU, comp], pt[0:fw, 0:TP]
                )
        nc.sync.dma_start(y_ap[f0:f0 + fw].rearrange("f t c -> f (t c)"), yo[:])


def _build():
    if "nc" in _CACHE:
        return _CACHE["nc"]
    from contextlib import ExitStack
    from concourse import bacc, mybir
    import concourse.tile as tile

    f32 = mybir.dt.float32
    nc = bacc.Bacc("TRN2", target_bir_lowering=False, debug=False, num_devices=B)
    m_d = nc.dram_tensor("m", (C, T, F), f32, kind="ExternalInput")
    x_d = nc.dram_tensor("x", (F, T, 2), f32, kind="ExternalInput")
    id_d = nc.dram_tensor("ident", (128, 128), f32, kind="ExternalInput")
    y_d = nc.dram_tensor("y", (F, T, 2), f32, kind="ExternalOutput")

    with tile.TileContext(nc) as tc:
        with ExitStack() as ctx:
            _emit(ctx, tc, m_d.ap(), x_d.ap(), id_d.ap(), y_d.ap())
    nc.compile()
    _CACHE["nc"] = nc
    return nc


def _in_maps(m, x):
    ident = np.eye(128, dtype=np.float32)
    return [
        {"m": np.ascontiguousarray(m[b]), "x": np.ascontiguousarray(x[b]),
         "ident": ident}
        for b in range(B)
    ]


def kernel(m, x, v, _trace=False):
    from concourse import bass_utils

    m = np.asarray(m, dtype=np.float32)
    x = np.asarray(x, dtype=np.float32)
    nc = _build()
    res = bass_utils.run_bass_kernel_spmd(
        nc, _in_maps(m, x), core_ids=list(range(B)), trace=_trace
    )
    kernel.last_results = res
    y = np.stack([res.results[b]["y"] for b in range(B)], axis=0)
    return y



# revision 3
# speedup vs baseline: 1.2622x; 1.2622x over previous
"""Trainium2 Bass kernel for nn_CCM: per-pixel complex 3x3 conv mask.

Math (per batch element b, sharded 1 batch element per NeuronCore):
  y[t,f] = sum_{n=0..8} A_n[t,f] * X[t+i(n)-2, f+j(n)-1]   (complex)
with A_n = m_n + w * m_{9+n} + conj(w) * m_{18+n}, w = -1/2 + i*sqrt(3)/2:
  Ar_n = m_n - 0.5*(m_{9+n} + m_{18+n})
  Ai_n = s * (m_{9+n} - m_{18+n}),  s = sqrt(3)/2
X = xr + i*xi, zero padded (causal in t: 2 top; symmetric in f: 1,1).

Implementation notes:
- bf16 everywhere on the DVE so tensor_tensor ops run in 2x_1p mode.
- Layout: t = 8*p + tau, partitions p in [0,125); free dim is flat
  (slot, f-col) with slot row width 260 (even, keeps bf16 reads 4B-aligned).
- x planes stored twice (A: f origin at col 1, B: f origin at col 0) so all
  three f-shifts read at even element offsets.
- m tiles: k=0 third loaded via gpsimd SWDGE casting DMA (fp32->bf16 in
  flight); k=1,2 thirds loaded fp32 via the two HWDGE rings (sync/scalar)
  and converted on the Scalar engine with the -0.5 scale folded in.
"""

import sys
import numpy as np

sys.path.insert(0, "/opt/trn_rl_repo")

B = 8
C = 27
T = 1000
F = 257
TP = 125          # partitions
TAU = 8           # t = 8*p + tau
NS = 10           # slots in x planes: tau in [-2, 8)
SROW = 260        # slot row width (elements)
MW = TAU * SROW   # 2080: m / acc tile width
PLW = NS * SROW + 4   # 2604: x plane width (pad 4 so +2-offset reads stay in)
SQ3H = float(np.sqrt(3.0) / 2.0)

_CACHE = {}


def _emit(ctx, tc, m_ap, x_ap, id_ap, y_ap):
    import concourse.mybir as mybir

    nc = tc.nc
    f32 = mybir.dt.float32
    bf16 = mybir.dt.bfloat16
    FCS = [(0, 128), (128, 128), (256, 1)]   # f chunks for transposes
    SLOT_GROUPS = [(0, 4), (4, 4), (8, 2)]   # batches of slots per psum tile

    const = ctx.enter_context(tc.tile_pool(name="const", bufs=1))
    mfpool = ctx.enter_context(tc.tile_pool(name="mf", bufs=4))
    mhpool = ctx.enter_context(tc.tile_pool(name="mh", bufs=4))
    mbpool = ctx.enter_context(tc.tile_pool(name="mb0", bufs=3))
    prep = ctx.enter_context(tc.tile_pool(name="prep", bufs=2))
    prod = ctx.enter_context(tc.tile_pool(name="prod", bufs=3))
    yop = ctx.enter_context(tc.tile_pool(name="yop", bufs=2))
    psum = ctx.enter_context(tc.tile_pool(name="psum", bufs=3, space="PSUM"))
    psum2 = ctx.enter_context(tc.tile_pool(name="psum2", bufs=3, space="PSUM"))

    ident = const.tile([128, 128], f32, tag="ident")
    nc.sync.dma_start(ident[:], id_ap)
    identb = const.tile([128, 128], bf16, tag="identb")
    nc.scalar.copy(identb[:], ident[:])

    # ---- SWDGE casting loads for the k=0 tiles (c = 0..8), fp32 -> bf16
    mb0s = []
    for n in range(9):
        mb = mbpool.tile([TP, MW], bf16, tag="mb0")
        mbv = mb.rearrange("p (r w) -> p r w", w=SROW)
        nc.gpsimd.dma_start(
            mbv[:, :, 0:F], m_ap[n].rearrange("(p t) f -> p t f", p=TP)
        )
        mb0s.append(mb)

    # ---- load x in natural layout: [f, (tt, comp)] with tt = t + 2
    xns = []
    for (f0, fw) in FCS:
        xn = const.tile([fw, (T + 2) * 2], f32, tag=f"xn{f0}")
        nc.vector.memset(xn[:, 0:4], 0.0)
        nc.sync.dma_start(
            xn[:, 4:], x_ap[f0:f0 + fw].rearrange("f t c -> f (t c)")
        )
        xns.append(xn)

    # ---- x planes (bf16): A has f origin at col 1, B at col 0
    planes = {}
    for nm in ("xrA", "xiA", "xrB", "xiB"):
        p = const.tile([TP, PLW], bf16, tag=nm)
        pv = p[:, 0:NS * SROW].rearrange("p (s w) -> p s w", w=SROW)
        if nm.endswith("A"):
            nc.vector.memset(pv[:, :, 0:1], 0.0)
            nc.vector.memset(pv[:, :, 258:260], 0.0)
        else:
            nc.vector.memset(pv[:, :, 257:260], 0.0)
        nc.vector.memset(p[:, NS * SROW:], 0.0)
        planes[nm] = p

    # ---- transpose x into the planes: PE (grouped into psum) + ACT copies
    for q, (nmA, nmB) in enumerate((("xrA", "xrB"), ("xiA", "xiB"))):
        pA = planes[nmA][:, 0:NS * SROW].rearrange("p (s w) -> p s w", w=SROW)
        pB = planes[nmB][:, 0:NS * SROW].rearrange("p (s w) -> p s w", w=SROW)
        for ci, (f0, fw) in enumerate(FCS):
            xn3 = xns[ci].rearrange("f (t c) -> f t c", c=2)
            for (g0, gn) in SLOT_GROUPS:
                ptg = psum.tile([TP, 512], f32, tag="ptg")
                for u in range(gn):
                    ts = g0 + u
                    nc.tensor.transpose(
                        ptg[0:TP, 128 * u:128 * u + fw],
                        xn3[0:fw, ts:ts + TAU * (TP - 1) + 1:TAU, q],
                        ident[0:fw, 0:fw],
                    )
                src = ptg.rearrange("p (u w) -> p u w", w=128)[0:TP, 0:gn, 0:fw]
                nc.scalar.copy(pA[:, g0:g0 + gn, 1 + f0:1 + f0 + fw], src)
                nc.scalar.copy(pB[:, g0:g0 + gn, f0:f0 + fw], src)

    # ---- HWDGE fp32 loads for k=1,2 tiles + ACT convert with -0.5 scale
    mhs = {}
    for n in range(9):
        for c, eng, tg in ((9 + n, nc.sync, "s"), (18 + n, nc.scalar, "a")):
            mf = mfpool.tile([TP, TAU * F], f32, tag=f"mf{tg}", bufs=2)
            eng.dma_start(mf[:], m_ap[c].rearrange("(p t) f -> p (t f)", p=TP))
            mh = mhpool.tile([TP, MW], bf16, tag=f"mh{tg}", bufs=2)
            nc.scalar.mul(
                mh.rearrange("p (r w) -> p r w", w=SROW)[:, :, 0:F],
                mf.rearrange("p (r w) -> p r w", w=F),
                -0.5,
            )
            mhs[c] = mh

    # ---- tap loop
    mult = mybir.AluOpType.mult
    accr = const.tile([TP, MW], bf16, tag="accr")
    acci = const.tile([TP, MW], bf16, tag="acci")
    for n in range(C // 3):
        i, j = divmod(n, 3)
        xrP = planes["xrB"] if j == 1 else planes["xrA"]
        xiP = planes["xiB"] if j == 1 else planes["xiA"]
        off = i * SROW + (2 if j == 2 else 0)
        xrv = xrP[:, off:off + MW]
        xiv = xiP[:, off:off + MW]
        mb0, mh9, mh18 = mb0s[n], mhs[9 + n], mhs[18 + n]

        t1 = prep.tile([TP, MW], bf16, tag="t1", bufs=1)
        nc.vector.tensor_add(t1[:], mh9[:], mh18[:])
        ar = prep.tile([TP, MW], bf16, tag="ar", bufs=2)
        nc.vector.tensor_add(ar[:], t1[:], mb0[:])
        d = prep.tile([TP, MW], bf16, tag="d", bufs=1)
        nc.vector.tensor_sub(d[:], mh9[:], mh18[:])
        ds = prep.tile([TP, MW], bf16, tag="ds", bufs=2)
        nc.vector.tensor_scalar_mul(ds[:], d[:], -2.0 * SQ3H)  # = Ai_n

        if n == 0:
            nc.vector.tensor_mul(accr[:], ar[:], xrv)
            nc.vector.tensor_mul(acci[:], ar[:], xiv)
        else:
            p0 = prod.tile([TP, MW], bf16, tag="prod")
            nc.vector.tensor_mul(p0[:], ar[:], xrv)
            nc.vector.tensor_add(accr[:], accr[:], p0[:])
            p1 = prod.tile([TP, MW], bf16, tag="prod")
            nc.vector.tensor_mul(p1[:], ar[:], xiv)
            nc.vector.tensor_add(acci[:], acci[:], p1[:])
        p2 = prod.tile([TP, MW], bf16, tag="prod")
        nc.vector.tensor_mul(p2[:], ds[:], xiv)
        nc.vector.tensor_sub(accr[:], accr[:], p2[:])
        p3 = prod.tile([TP, MW], bf16, tag="prod")
        nc.vector.tensor_mul(p3[:], ds[:], xrv)
        nc.vector.tensor_add(acci[:], acci[:], p3[:])

    # ---- transpose back to [f, (t, comp)] and store
    for ci, (f0, fw) in enumerate(FCS):
        yo = yop.tile([128, T * 2], f32, tag="yo")
        yv = yo.rearrange("f (t c) -> f t c", c=2)
        for comp, acc in ((0, accr), (1, acci)):
            accv = acc.rearrange("p (r w) -> p r w", w=SROW)
            for r in range(TAU):
                pt2 = psum2.tile([128, TP], bf16, tag="pt2")
                nc.tensor.transpose(
                    pt2[0:fw, 0:TP], accv[:, r, f0:f0 + fw], identb[0:TP, 0:TP]
                )
                nc.scalar.copy(
                    yv[0:fw, r:r + TAU * (TP - 1) + 1:TAU, comp],
                    pt2[0:fw, 0:TP],
                )
        nc.sync.dma_start(
            y_ap[f0:f0 + fw].rearrange("f t c -> f (t c)"), yo[0:fw, :]
        )


def _build():
    if "nc" in _CACHE:
        return _CACHE["nc"]
    from contextlib import ExitStack
    from concourse import bacc, mybir
    import concourse.tile as tile

    f32 = mybir.dt.float32
    nc = bacc.Bacc("TRN2", target_bir_lowering=False, debug=False, num_devices=B)
    m_d = nc.dram_tensor("m", (C, T, F), f32, kind="ExternalInput")
    x_d = nc.dram_tensor("x", (F, T, 2), f32, kind="ExternalInput")
    id_d = nc.dram_tensor("ident", (128, 128), f32, kind="ExternalInput")
    y_d = nc.dram_tensor("y", (F, T, 2), f32, kind="ExternalOutput")

    with tile.TileContext(nc) as tc:
        with ExitStack() as ctx:
            _emit(ctx, tc, m_d.ap(), x_d.ap(), id_d.ap(), y_d.ap())
    nc.compile()
    _CACHE["nc"] = nc
    return nc


def _in_maps(m, x):
    ident = np.eye(128, dtype=np.float32)
    return [
        {"m": np.ascontiguousarray(m[b]), "x": np.ascontiguousarray(x[b]),
         "ident": ident}
        for b in range(B)
    ]


def kernel(m, x, v, _trace=False):
    from concourse import bass_utils

    m = np.asarray(m, dtype=np.float32)
    x = np.asarray(x, dtype=np.float32)
    nc = _build()
    res = bass_utils.run_bass_kernel_spmd(
        nc, _in_maps(m, x), core_ids=list(range(B)), trace=_trace
    )
    kernel.last_results = res
    y = np.stack([res.results[b]["y"] for b in range(B)], axis=0)
    return y


# revision 6
# speedup vs baseline: 1.6728x; 1.3253x over previous
"""Trainium2 Bass kernel for nn_CCM: per-pixel complex 3x3 conv mask.

Math (per batch element b, sharded 1 batch element per NeuronCore):
  y[t,f] = sum_{n=0..8} A_n[t,f] * X[t+i(n)-2, f+j(n)-1]   (complex)
with A_n = m_n + w * m_{9+n} + conj(w) * m_{18+n}, w = -1/2 + i*sqrt(3)/2:
  Ar_n = m_n - 0.5*(m_{9+n} + m_{18+n})
  Ai_n = s * (m_{9+n} - m_{18+n}),  s = sqrt(3)/2
X = xr + i*xi, zero padded (causal in t: 2 top; symmetric in f: 1,1).

Implementation notes:
- bf16 everywhere on the DVE so tensor_tensor ops run in 2x_1p mode.
- Layout: t = 8*p + tau, partitions p in [0,125); free dim is flat
  (slot, f-col) with slot row width 260 (even, keeps bf16 reads 4B-aligned).
- x planes stored twice (A: f origin at col 1, B: f origin at col 0) so all
  three f-shifts read at even element offsets.
- m tiles: k=0 third loaded via gpsimd SWDGE casting DMA (fp32->bf16 in
  flight); k=1,2 thirds loaded fp32 via the two HWDGE rings (sync/scalar)
  and converted on the Scalar engine with the -0.5 scale folded in.
"""

import sys
import numpy as np

sys.path.insert(0, "/opt/trn_rl_repo")

B = 8
C = 27
T = 1000
F = 257
TP = 125          # partitions
TAU = 8           # t = 8*p + tau
NS = 10           # slots in x planes: tau in [-2, 8)
SROW = 260        # slot row width (elements)
MW = TAU * SROW   # 2080: m / acc tile width
PLW = NS * SROW + 4   # 2604: x plane width (pad 4 so +2-offset reads stay in)
SQ3H = float(np.sqrt(3.0) / 2.0)

_CACHE = {}


def _emit(ctx, tc, m_ap, x_ap, id_ap, y_ap):
    import concourse.mybir as mybir

    nc = tc.nc
    f32 = mybir.dt.float32
    bf16 = mybir.dt.bfloat16
    FCS = [(0, 128), (128, 128), (256, 1)]   # f chunks for transposes
    SLOT_GROUPS = [(0, 4), (4, 4), (8, 2)]   # batches of slots per psum tile

    const = ctx.enter_context(tc.tile_pool(name="const", bufs=1))
    mcpool = ctx.enter_context(tc.tile_pool(name="mc", bufs=8))
    mhpool = ctx.enter_context(tc.tile_pool(name="mh", bufs=6))
    prep = ctx.enter_context(tc.tile_pool(name="prep", bufs=2))
    prod = ctx.enter_context(tc.tile_pool(name="prod", bufs=3))
    yop = ctx.enter_context(tc.tile_pool(name="yop", bufs=2))
    psum = ctx.enter_context(tc.tile_pool(name="psum", bufs=3, space="PSUM"))
    psum2 = ctx.enter_context(tc.tile_pool(name="psum2", bufs=3, space="PSUM"))

    ident = const.tile([128, 128], f32, tag="ident")
    nc.sync.dma_start(ident[:], id_ap)
    identb = const.tile([128, 128], bf16, tag="identb")
    nc.scalar.copy(identb[:], ident[:])

    # ---- all m tiles: SWDGE casting DMA fp32 -> bf16 into flat tiles
    # (contiguous per-partition runs keep descriptors big), then ACT
    # restride to the 260-wide row layout, folding the -0.5 basis scale
    # into the copy for the k=1,2 thirds.
    def load_m(c, scale):
        mc = mcpool.tile([TP, TAU * F], bf16, tag="mc")
        nc.gpsimd.dma_start(
            mc[:], m_ap[c].rearrange("(p t) f -> p (t f)", p=TP)
        )
        mh = mhpool.tile([TP, MW], bf16, tag="mh")
        dst = mh.rearrange("p (r w) -> p r w", w=SROW)[:, :, 0:F]
        src = mc.rearrange("p (r w) -> p r w", w=F)
        if scale == 1.0:
            nc.scalar.copy(dst, src)
        else:
            nc.scalar.mul(dst, src, scale)
        return mh

    mtiles = {}
    for n in range(9):
        for c in (n, 9 + n, 18 + n):
            mtiles[c] = load_m(c, 1.0 if c < 9 else -0.5)

    # ---- load x in natural layout: [f, (tt, comp)] with tt = t + 2
    xns = []
    for (f0, fw) in FCS:
        xn = const.tile([fw, (T + 2) * 2], f32, tag=f"xn{f0}")
        nc.vector.memset(xn[:, 0:4], 0.0)
        nc.sync.dma_start(
            xn[:, 4:], x_ap[f0:f0 + fw].rearrange("f t c -> f (t c)")
        )
        xns.append(xn)

    # ---- x planes (bf16): A has f origin at col 1, B at col 0
    planes = {}
    for nm in ("xrA", "xiA", "xrB", "xiB"):
        p = const.tile([TP, PLW], bf16, tag=nm)
        pv = p[:, 0:NS * SROW].rearrange("p (s w) -> p s w", w=SROW)
        if nm.endswith("A"):
            nc.vector.memset(pv[:, :, 0:1], 0.0)
            nc.vector.memset(pv[:, :, 258:260], 0.0)
        else:
            nc.vector.memset(pv[:, :, 257:260], 0.0)
        nc.vector.memset(p[:, NS * SROW:], 0.0)
        planes[nm] = p

    # ---- transpose x into the planes: PE (grouped into psum) + ACT copies
    for q, (nmA, nmB) in enumerate((("xrA", "xrB"), ("xiA", "xiB"))):
        pA = planes[nmA][:, 0:NS * SROW].rearrange("p (s w) -> p s w", w=SROW)
        pB = planes[nmB][:, 0:NS * SROW].rearrange("p (s w) -> p s w", w=SROW)
        for ci, (f0, fw) in enumerate(FCS):
            xn3 = xns[ci].rearrange("f (t c) -> f t c", c=2)
            for (g0, gn) in SLOT_GROUPS:
                ptg = psum.tile([TP, 512], f32, tag="ptg")
                for u in range(gn):
                    ts = g0 + u
                    nc.tensor.transpose(
                        ptg[0:TP, 128 * u:128 * u + fw],
                        xn3[0:fw, ts:ts + TAU * (TP - 1) + 1:TAU, q],
                        ident[0:fw, 0:fw],
                    )
                src = ptg.rearrange("p (u w) -> p u w", w=128)[0:TP, 0:gn, 0:fw]
                nc.scalar.copy(pA[:, g0:g0 + gn, 1 + f0:1 + f0 + fw], src)
                nc.scalar.copy(pB[:, g0:g0 + gn, f0:f0 + fw], src)

    # ---- tap loop
    mult = mybir.AluOpType.mult
    accr = const.tile([TP, MW], bf16, tag="accr")
    acci = const.tile([TP, MW], bf16, tag="acci")
    for n in range(C // 3):
        i, j = divmod(n, 3)
        xrP = planes["xrB"] if j == 1 else planes["xrA"]
        xiP = planes["xiB"] if j == 1 else planes["xiA"]
        off = i * SROW + (2 if j == 2 else 0)
        xrv = xrP[:, off:off + MW]
        xiv = xiP[:, off:off + MW]
        mb0, mh9, mh18 = mtiles[n], mtiles[9 + n], mtiles[18 + n]

        t1 = prep.tile([TP, MW], bf16, tag="t1", bufs=1)
        nc.vector.tensor_add(t1[:], mh9[:], mh18[:])
        ar = prep.tile([TP, MW], bf16, tag="ar", bufs=2)
        nc.vector.tensor_add(ar[:], t1[:], mb0[:])
        d = prep.tile([TP, MW], bf16, tag="d", bufs=1)
        nc.vector.tensor_sub(d[:], mh9[:], mh18[:])
        ds = prep.tile([TP, MW], bf16, tag="ds", bufs=2)
        nc.vector.tensor_scalar_mul(ds[:], d[:], -2.0 * SQ3H)  # = Ai_n

        if n == 0:
            nc.vector.tensor_mul(accr[:], ar[:], xrv)
            nc.vector.tensor_mul(acci[:], ar[:], xiv)
        else:
            p0 = prod.tile([TP, MW], bf16, tag="prod")
            nc.vector.tensor_mul(p0[:], ar[:], xrv)
            nc.vector.tensor_add(accr[:], accr[:], p0[:])
            p1 = prod.tile([TP, MW], bf16, tag="prod")
            nc.vector.tensor_mul(p1[:], ar[:], xiv)
            nc.vector.tensor_add(acci[:], acci[:], p1[:])
        p2 = prod.tile([TP, MW], bf16, tag="prod")
        nc.vector.tensor_mul(p2[:], ds[:], xiv)
        nc.vector.tensor_sub(accr[:], accr[:], p2[:])
        p3 = prod.tile([TP, MW], bf16, tag="prod")
        nc.vector.tensor_mul(p3[:], ds[:], xrv)
        nc.vector.tensor_add(acci[:], acci[:], p3[:])

    # ---- transpose back to [f, (t, comp)] and store
    for ci, (f0, fw) in enumerate(FCS):
        yo = yop.tile([128, T * 2], f32, tag="yo")
        yv = yo.rearrange("f (t c) -> f t c", c=2)
        for comp, acc in ((0, accr), (1, acci)):
            accv = acc.rearrange("p (r w) -> p r w", w=SROW)
            for r in range(TAU):
                pt2 = psum2.tile([128, TP], bf16, tag="pt2")
                nc.tensor.transpose(
                    pt2[0:fw, 0:TP], accv[:, r, f0:f0 + fw], identb[0:TP, 0:TP]
                )
                nc.scalar.copy(
                    yv[0:fw, r:r + TAU * (TP - 1) + 1:TAU, comp],
                    pt2[0:fw, 0:TP],
                )
        nc.sync.dma_start(
            y_ap[f0:f0 + fw].rearrange("f t c -> f (t c)"), yo[0:fw, :]
        )


def _build():
    if "nc" in _CACHE:
        return _CACHE["nc"]
    from contextlib import ExitStack
    from concourse import bacc, mybir
    import concourse.tile as tile

    f32 = mybir.dt.float32
    nc = bacc.Bacc("TRN2", target_bir_lowering=False, debug=False, num_devices=B)
    m_d = nc.dram_tensor("m", (C, T, F), f32, kind="ExternalInput")
    x_d = nc.dram_tensor("x", (F, T, 2), f32, kind="ExternalInput")
    id_d = nc.dram_tensor("ident", (128, 128), f32, kind="ExternalInput")
    y_d = nc.dram_tensor("y", (F, T, 2), f32, kind="ExternalOutput")

    with tile.TileContext(nc) as tc:
        with ExitStack() as ctx:
            _emit(ctx, tc, m_d.ap(), x_d.ap(), id_d.ap(), y_d.ap())
    nc.compile()
    _CACHE["nc"] = nc
    return nc


def _in_maps(m, x):
    ident = np.eye(128, dtype=np.float32)
    return [
        {"m": np.ascontiguousarray(m[b]), "x": np.ascontiguousarray(x[b]),
         "ident": ident}
        for b in range(B)
    ]


def kernel(m, x, v, _trace=False):
    from concourse import bass_utils

    m = np.asarray(m, dtype=np.float32)
    x = np.asarray(x, dtype=np.float32)
    nc = _build()
    res = bass_utils.run_bass_kernel_spmd(
        nc, _in_maps(m, x), core_ids=list(range(B)), trace=_trace
    )
    kernel.last_results = res
    y = np.stack([res.results[b]["y"] for b in range(B)], axis=0)
    return y
